# revision 29
# baseline (speedup 1.0000x reference)
"""AutoRec forward pass on 8 Trainium2 NeuronCores (SPMD, no collectives).

Computation (reference):
    z = segment_sum(r[:,None] * V[u], i, num_segments=m)   # (m, D) sparse spmm
    h = sigmoid(z + mu)
    out = sum(h[j] * W[v], -1) + b[v]                      # (n_out,)

v4.4 strategy (v3 baseline was 3.71ms HW; this runs 3.23ms, rel err 1.7e-5).
Cost model established by probing: each dma_gather row costs one SDMA
descriptor (~2.5ns/row effective across 4 SWDGE queues, engine-serial
desc-gen is nearly free, row BYTES are free up to 512B) plus ~0.4us fixed
per instruction; so minimize descriptors and instructions, and never let
multi-packet spans outrun the SCRATCH/256-descs-per-lane ring (that stalls
the Q7 mid-instruction -- measured 2-3x blowups).
  - Users range-sharded over 8 cores (12500 each, 98 tiles of 128).
  - Phase 1 (z/h): edges bucketed (core, user-tile, item-chunk of 25000),
    bucket capacity = 128-ceil of max count across cores (SPMD-static).
    Gathers are 1024-idx spans (64 descs/lane = one HW packet,
    single_packet=True, SCRATCH=65536 so 4 spans/queue ride the ring)
    crossing bucket boundaries inside each (supergroup, chunk) arena:
    ~650 instructions, pad slots gather row 0 and are masked by rel=-1.
  - S scatter operands built with TWO broadcast-AP DVE ops per (sg, chunk)
    arena ([128, nblk, 128] stride-0 views): 224 DVE ops vs 4704 per-block
    tensor_scalars in v3 (DVE fixed cost ~0.5us/op dominated).
  - h is stored as h' = tanh((z+mu)/2) = sigmoid(z+mu) - 0.5, in bf16: the
    informative part of h survives bf16 (h ~ 0.5 would eat 8 mantissa bits;
    this is the main accuracy win, 2.9e-3 -> 1.7e-5).  One ACT op per tile.
    Phase 2 uses W/2 as its table and the host adds 0.5*rowsum(W)[v] + b[v]
    after unshard.
  - Phase 2 (decode): pairs sorted by (core, supergroup, item-segment of
    32768, user-tile).  h'[j] is NOT gathered: each 128-pair block contracts
    a one-hot S2 against the SBUF-resident h' tile of its user tile via PE
    (lhsT = S2 slice, rhs = h' tile -> PSUM), removing 250k SWDGE
    descriptors/core.  S2 is built in 512-col chunks: a rank-1 PE matmul
    (ones^T @ jrel row) replicates jrel into PSUM (no 85MB broadcast DMA)
    and DVE is_eq's it against a partition-iota.  W[v]/2 is span-gathered
    per (supergroup, segment) region; hg is copied PSUM->SBUF bf16 on the
    idle ACT engine so the DVE mult runs dense bf16; dot = merged mult +
    segmented tensor_reduce(axis=X).
  - SWDGE queues: 4 (ucode max), queue = sem-lane % 4 assigned post-schedule
    (sems lock to their first queue; lane rotation is stable in final order).
  - Remaining wall (measured): ~2.7ms Pool-engine gather occupancy
    (desc-bound floor ~2.3ms for 912k rows/core incl padding) + ~0.4ms
    phase-2 pacing (DMASW sem-lane reuse throttles gathers to ~3 regions of
    lookahead).  Deeper fixes tried and rejected: mid-span -1 idx skipping
    (2.6x slower), fp8 rows (bytes are free, descriptors are not),
    num_idxs_reg runtime counts (device crash), sg-interleaved phases (PSUM
    bank conflict forces 1-buf hg, net slower), 1792+ spans with
    single_packet=False (ring-drain stalls + 20-30% run-to-run variance).
"""

import math
import sys

sys.path.insert(0, "/opt/trn_rl_repo")

import numpy as np
import ml_dtypes

D = 128
M_CORES = 8
NI = 200000
NU = 100000
NCH = 8                     # phase-1 item chunks (int16 gather index range)
CHUNK = NI // NCH           # 25000 < 32768
UC = (NU + M_CORES - 1) // M_CORES        # users per core = 12500
T1 = (UC + 127) // 128                    # user tiles per core = 98
SG = 7                      # user tiles per supergroup (7 PSUM banks)
NSG = T1 // SG              # 14
SEGW = 32768                # phase-2 item segment (int16 range)
NSEG = (NI + SEGW - 1) // SEGW            # 7
NQ = 4                      # SWDGE queues, round-robin
SPAN = 1024                 # max idx per dma_gather: 64 descs/lane = one HW
                            # packet (single_packet mode); ring is SCRATCH/256
                            # = 256 descs/lane/queue, so 4 spans fit in flight
SCRATCH = 65536             # SWDGE descriptor-ring carveout (bytes/partition)

_PROGRAM_CACHE: dict = {}


def _build_program(shapes):
    """shapes: static capacity tables derived from data (same on all cores)."""
    import concourse.bacc as bacc
    import concourse.mybir as mybir
    from concourse.tile import TileContext

    f32 = mybir.dt.float32
    bf16 = mybir.dt.bfloat16
    i16 = mybir.dt.int16
    i32 = mybir.dt.int32
    ALU = mybir.AluOpType
    ACT = mybir.ActivationFunctionType

    capr = np.asarray(shapes["capr"]).reshape(NSG, SG, NCH)  # phase-1 slots
    cap2 = np.asarray(shapes["cap2"]).reshape(NSG, NSEG, SG)  # phase-2 slots
    Br = capr >> 7
    # phase-1 per-(sg,ch) arena: blocks of 7 tiles concatenated, ch-major
    SGB = Br.sum(axis=1)                    # [NSG, NCH] blocks per (sg, ch)
    ga = np.zeros((NSG, NCH + 1), np.int64)
    np.cumsum(SGB, axis=1, out=ga[:, 1:])
    NBLK = ga[:, NCH]
    NBLKmax = int(NBLK.max())
    SGBmax = int(SGB.max())
    gboff = np.zeros((NSG, NCH, SG + 1), np.int64)
    np.cumsum(np.swapaxes(Br, 1, 2), axis=2, out=gboff[:, :, 1:])
    # phase-2 per-(sg,seg) regions
    B2 = cap2 >> 7                          # [NSG, NSEG, SG]
    RS = cap2.sum(axis=2)                   # region slots [NSG, NSEG]
    RB = B2.sum(axis=2)                     # region blocks
    tloff = np.zeros((NSG, NSEG, SG + 1), np.int64)
    np.cumsum(cap2, axis=2, out=tloff[:, :, 1:])
    regoff = np.zeros(NSG * NSEG + 1, np.int64)
    np.cumsum(RS.reshape(-1), out=regoff[1:])
    regoff = regoff.reshape(-1)
    regblk = np.zeros(NSG * NSEG + 1, np.int64)
    np.cumsum(RB.reshape(-1), out=regblk[1:])
    TOT2S = int(regoff[-1])
    TOT2B = int(regblk[-1])
    RSmax = int(RS.max())
    RBmax = int(RB.max())
    # block -> tile map per (sg, seg)
    tob = {}
    for sg in range(NSG):
        for seg in range(NSEG):
            lst = []
            for tl in range(SG):
                lst += [tl] * int(B2[sg, seg, tl])
            tob[(sg, seg)] = lst

    nc = bacc.Bacc("TRN2", target_bir_lowering=False, debug=False,
                   num_swdge_queues=NQ, dynamic_dma_scratch_size=SCRATCH)

    V_d = nc.dram_tensor("V", [NI, D], bf16, kind="ExternalInput")
    W_d = nc.dram_tensor("W", [NI, D], bf16, kind="ExternalInput")  # = W/2
    mu_d = nc.dram_tensor("mu", [1, D], f32, kind="ExternalInput")
    eidx_d = nc.dram_tensor("eidx", [NSG, 128, NBLKmax * 8], i16,
                            kind="ExternalInput")
    erel_d = nc.dram_tensor("erel", [NSG, 128, NBLKmax], bf16,
                            kind="ExternalInput")
    erat_d = nc.dram_tensor("erat", [NSG, 128, NBLKmax], bf16,
                            kind="ExternalInput")
    pv_d = nc.dram_tensor("pv", [128, TOT2S // 16], i16, kind="ExternalInput")
    pjr_d = nc.dram_tensor("pjr", [1, TOT2S], bf16, kind="ExternalInput")
    res_d = nc.dram_tensor("res", [128, TOT2B], f32, kind="ExternalOutput")
    h_d = nc.dram_tensor("hscratch", [T1 * 128, D], bf16, kind="Internal")

    with TileContext(nc) as tc:
        with tc.tile_pool(name="const", bufs=1) as constp:
            iota_i = constp.tile([128, 128], i32)
            nc.gpsimd.iota(iota_i[:], pattern=[[1, 128]], base=0,
                           channel_multiplier=0)
            iota_bf = constp.tile([128, 128], bf16)
            nc.vector.tensor_copy(iota_bf[:], iota_i[:])
            iotac_i = constp.tile([128, 1], i32)
            nc.gpsimd.iota(iotac_i[:], pattern=[[1, 1]], base=0,
                           channel_multiplier=1)
            iotac_bf = constp.tile([128, 1], bf16)
            nc.vector.tensor_copy(iotac_bf[:], iotac_i[:])
            ones1 = constp.tile([1, 128], f32)
            nc.vector.memset(ones1[:], 1.0)
            ones1b = constp.tile([1, 128], bf16)
            nc.vector.memset(ones1b[:], 1.0)
            mu_t = constp.tile([1, 128], f32)
            nc.sync.dma_start(out=mu_t[:], in_=mu_d[:])

            # -------- phase 1: h' = tanh((z + mu)/2) -> h_d
            with tc.tile_pool(name="p1meta", bufs=3) as mp, \
                 tc.tile_pool(name="p1g", bufs=4) as gp, \
                 tc.tile_pool(name="p1s", bufs=2) as sp, \
                 tc.tile_pool(name="p1h", bufs=4) as hp, \
                 tc.tile_pool(name="p1z", bufs=1, space="PSUM") as zp:
                for sg in range(NSG):
                    it = mp.tile([128, NBLKmax * 8], i16, tag="it")
                    nc.sync.dma_start(out=it[:], in_=eidx_d[sg])
                    rel = mp.tile([128, NBLKmax], bf16, tag="rel")
                    nc.sync.dma_start(out=rel[:], in_=erel_d[sg])
                    rat = mp.tile([128, NBLKmax], bf16, tag="rat")
                    nc.sync.dma_start(out=rat[:], in_=erat_d[sg])
                    zts = []
                    last_ch = [max(c for c in range(NCH) if Br[sg, tl, c] > 0)
                               for tl in range(SG)]
                    for tl in range(SG):
                        zt = zp.tile([128, 128], f32, tag=f"zt{tl}")
                        nc.tensor.matmul(zt[:], lhsT=ones1[:], rhs=mu_t[:],
                                         start=True, stop=False)
                        zts.append(zt)
                    for ch in range(NCH):
                        nblk = int(SGB[sg, ch])
                        gac = int(ga[sg, ch])
                        tot = nblk * 128
                        g = gp.tile([128, SGBmax * 128], bf16, tag="g")
                        for s0 in range(0, tot, SPAN):
                            n = min(SPAN, tot - s0)
                            nc.gpsimd.dma_gather(
                                g[:, s0:s0 + n].rearrange(
                                    "p (b d) -> p b d", d=D),
                                V_d[ch * CHUNK:min(NI, (ch + 1) * CHUNK)],
                                it[:, gac * 8 + s0 // 16:
                                   gac * 8 + (s0 + n) // 16],
                                n, n, D,
                                single_packet=True,
                            )
                        # merged S build for this (sg, ch) arena
                        S = sp.tile([128, SGBmax * 128], bf16, tag="S")
                        t1 = sp.tile([128, SGBmax * 128], bf16, tag="t1")
                        t1v = t1[:, :tot].rearrange("p (b s) -> p b s", s=128)
                        Sv = S[:, :tot].rearrange("p (b s) -> p b s", s=128)
                        relb = rel[:, gac:gac + nblk].unsqueeze(2).broadcast_to(
                            [128, nblk, 128])
                        ratb = rat[:, gac:gac + nblk].unsqueeze(2).broadcast_to(
                            [128, nblk, 128])
                        iob = iota_bf[:].unsqueeze(1).broadcast_to(
                            [128, nblk, 128])
                        nc.vector.tensor_tensor(out=t1v, in0=iob, in1=relb,
                                                op=ALU.is_equal)
                        nc.vector.tensor_tensor(out=Sv, in0=t1v, in1=ratb,
                                                op=ALU.mult)
                        for tl in range(SG):
                            nb = int(Br[sg, tl, ch])
                            b0 = int(gboff[sg, ch, tl])
                            for b in range(nb):
                                k = b0 + b
                                stop = (ch == last_ch[tl] and b == nb - 1)
                                nc.tensor.matmul(
                                    zts[tl][:],
                                    lhsT=S[:, k * 128:(k + 1) * 128],
                                    rhs=g[:, k * 128:(k + 1) * 128],
                                    start=False, stop=stop)
                    for tl in range(SG):
                        t = sg * SG + tl
                        hb = hp.tile([128, 128], bf16, tag="hb")
                        nc.scalar.activation(hb[:], zts[tl][:], ACT.Tanh,
                                             scale=0.5)
                        nc.sync.dma_start(
                            out=h_d[t * 128:(t + 1) * 128], in_=hb[:])

            # -------- phase 2: res[p, blk] = sum_d h'[j] * (W/2)[v]
            # jrel replication is a rank-1 PE matmul (ones^T @ jrow) into
            # PSUM -- no 85MB broadcast DMA; hg is copied PSUM->SBUF bf16 on
            # the otherwise-idle ACT engine so the DVE mult runs dense bf16.
            with tc.tile_pool(name="p2ht", bufs=2) as htp, \
                 tc.tile_pool(name="p2jr", bufs=2) as jrp, \
                 tc.tile_pool(name="p2iv", bufs=2) as ivp, \
                 tc.tile_pool(name="p2s2", bufs=3) as s2p, \
                 tc.tile_pool(name="p2wg", bufs=6) as wgp, \
                 tc.tile_pool(name="p2hc", bufs=6) as hcp, \
                 tc.tile_pool(name="p2pr", bufs=4) as prp, \
                 tc.tile_pool(name="p2rt", bufs=2) as rtp, \
                 tc.tile_pool(name="p2hg", bufs=3, space="PSUM") as hgp, \
                 tc.tile_pool(name="p2jp", bufs=2, space="PSUM") as jpp:
                sgoff = [int(regoff[sg * NSEG]) for sg in range(NSG + 1)]
                SGITV = max(sgoff[s + 1] - sgoff[s] for s in range(NSG)) // 16
                for sg in range(NSG):
                    hts = []
                    for tl in range(SG):
                        t = sg * SG + tl
                        ht = htp.tile([128, 128], bf16, tag=f"ht{tl}")
                        nc.sync.dma_start(out=ht[:],
                                          in_=h_d[t * 128:(t + 1) * 128])
                        hts.append(ht)
                    # merged per-sg loads of pair idx + jrel rows
                    sgs = sgoff[sg]
                    sgn = sgoff[sg + 1] - sgs
                    itv = ivp.tile([128, SGITV], i16, tag="itv")
                    nc.sync.dma_start(
                        out=itv[:, :sgn // 16],
                        in_=pv_d[:, sgs // 16:(sgs + sgn) // 16])
                    for seg in range(NSEG):
                        ridx = sg * NSEG + seg
                        ns = int(RS[sg, seg])
                        nbr = int(RB[sg, seg])
                        if nbr == 0:
                            continue
                        roff = int(regoff[ridx]) - sgs   # within sg tiles
                        rblk = int(regblk[ridx])
                        jrow = jrp.tile([1, RSmax], bf16, tag="jrow")
                        nc.sync.dma_start(
                            out=jrow[0:1, :ns],
                            in_=pjr_d[0:1, int(regoff[ridx]):
                                      int(regoff[ridx]) + ns])
                        # S2 one-hot built per 512-col chunk from a PSUM
                        # replica of the jrel row
                        S2 = s2p.tile([128, RSmax], bf16, tag="S2")
                        for c0 in range(0, ns, 512):
                            cn = min(512, ns - c0)
                            jp = jpp.tile([128, 512], f32, tag="jp")
                            nc.tensor.matmul(
                                jp[:, :cn], lhsT=ones1b[:],
                                rhs=jrow[0:1, c0:c0 + cn],
                                start=True, stop=True)
                            nc.vector.tensor_tensor(
                                out=S2[:, c0:c0 + cn],
                                in0=iotac_bf[:].broadcast_to([128, cn]),
                                in1=jp[:, :cn], op=ALU.is_equal)
                        wg = wgp.tile([128, RBmax * 128], bf16, tag="wg")
                        for s0 in range(0, ns, SPAN):
                            n = min(SPAN, ns - s0)
                            nc.gpsimd.dma_gather(
                                wg[:, s0:s0 + n].rearrange(
                                    "p (b d) -> p b d", d=D),
                                W_d[seg * SEGW:min(NI, (seg + 1) * SEGW)],
                                itv[:, (roff + s0) // 16:
                                    (roff + s0 + n) // 16],
                                n, n, D,
                                single_packet=True,
                            )
                        prod = prp.tile([128, RBmax * 128], bf16, tag="prod")
                        tobr = tob[(sg, seg)]
                        for b0 in range(0, nbr, 8):
                            nbb = min(8, nbr - b0)
                            hg = hgp.tile([128, 1024], f32, tag="hg")
                            for b in range(nbb):
                                nc.tensor.matmul(
                                    hg[:, b * 128:(b + 1) * 128],
                                    lhsT=S2[:, (b0 + b) * 128:
                                            (b0 + b + 1) * 128],
                                    rhs=hts[tobr[b0 + b]][:],
                                    start=True, stop=True)
                            hgc = hcp.tile([128, 1024], bf16, tag="hgc")
                            nc.scalar.activation(hgc[:, :nbb * 128],
                                                 hg[:, :nbb * 128], ACT.Copy)
                            nc.vector.tensor_tensor(
                                out=prod[:, b0 * 128:(b0 + nbb) * 128],
                                in0=hgc[:, :nbb * 128],
                                in1=wg[:, b0 * 128:(b0 + nbb) * 128],
                                op=ALU.mult)
                        rt = rtp.tile([128, RBmax], f32, tag="rt")
                        nc.vector.tensor_reduce(
                            out=rt[:, :nbr],
                            in_=prod[:, :nbr * 128].rearrange(
                                "p (b s) -> p b s", s=128),
                            axis=mybir.AxisListType.X, op=ALU.add)
                        nc.sync.dma_start(
                            out=res_d[:, rblk:rblk + nbr],
                            in_=rt[:, :nbr])

    # Post-schedule queue assignment: the tile scheduler rotates DMASW sem
    # lanes (8) over Pool DMA instructions in final program order, and each
    # sem is locked to the first SWDGE queue that uses it -- so queue must be
    # a stable function of the lane: queue = lane % NQ.
    import concourse.bass_isa as bass_isa
    import concourse.mybir as mybir2
    lane = 0
    for bb in nc.m.functions[0].blocks:
        for inst in bb.instructions:
            if isinstance(inst, bass_isa.AnyDMAInstruction) and \
                    inst.engine == mybir2.EngineType.Pool:
                if hasattr(inst, "queue_num"):
                    inst.queue_num = lane % NQ
                lane += 1

    nc.compile()
    return nc


def _pack16(a):
    """[..., N] int16 -> [..., 128, N//16]: element k at partition k%16,
    col k//16, replicated 8x across the 128 partitions."""
    lead = a.shape[:-1]
    n = a.shape[-1]
    x = a.reshape(*lead, n // 16, 16)
    x = np.moveaxis(x, -1, -2)                      # [..., 16, n//16]
    x = np.broadcast_to(x[..., None, :, :], (*lead, 8, 16, n // 16))
    return np.ascontiguousarray(x.reshape(*lead, 128, n // 16))


def _prep_inputs(u, i, r, m, v, j, V, mu, W, b):
    """Host-side sharding. Returns per-core input maps + unshard info."""
    bf = ml_dtypes.bfloat16
    u32 = np.asarray(u).astype(np.int32)
    i32 = np.asarray(i).astype(np.int32)
    r32 = np.asarray(r, dtype=np.float32)
    NNZ = u32.shape[0]

    # ---- phase 1 buckets: (core, tile, chunk)
    core = i32 // UC
    urel = i32 - core * UC
    trel = urel >> 7
    prel = (urel & 127).astype(np.float32)
    ch = u32 // CHUNK
    bkt = (core * T1 + trel) * NCH + ch
    NB1 = M_CORES * T1 * NCH
    order = np.argsort(bkt, kind="stable")
    bs = bkt[order]
    cnts = np.bincount(bkt, minlength=NB1)
    starts = np.zeros(NB1 + 1, np.int64)
    np.cumsum(cnts, out=starts[1:])
    pos = np.arange(NNZ, dtype=np.int64) - starts[bs]

    cnts3 = cnts.reshape(M_CORES, T1, NCH)
    cap = ((cnts3.max(0) + 127) // 128) * 128       # [T1, NCH] static slots
    capr = cap.reshape(NSG, SG, NCH)
    Br = capr >> 7
    SGB = Br.sum(axis=1)                            # [NSG, NCH]
    ga = np.zeros((NSG, NCH + 1), np.int64)
    np.cumsum(SGB, axis=1, out=ga[:, 1:])
    NBLK = ga[:, NCH]
    NBLKmax = int(NBLK.max())
    gboff = np.zeros((NSG, NCH, SG + 1), np.int64)
    np.cumsum(np.swapaxes(Br, 1, 2), axis=2, out=gboff[:, :, 1:])

    # per-(tile,chunk) global block base within its sg arena
    blkbase_tc = np.zeros((T1, NCH), np.int64)
    for sgi in range(NSG):
        for chi in range(NCH):
            for tli in range(SG):
                blkbase_tc[sgi * SG + tli, chi] = \
                    ga[sgi, chi] + gboff[sgi, chi, tli]

    eb = blkbase_tc[trel, ch]
    sgidx = trel // SG
    dslot = eb[order] * 128 + pos
    dcore = core[order]
    dsg = sgidx[order]

    EIDX = np.zeros((M_CORES, NSG, NBLKmax * 128), np.int16)
    EIDX[dcore, dsg, dslot] = (u32[order] % CHUNK).astype(np.int16)
    EREL = np.full((M_CORES, NSG, 128, NBLKmax), -1.0, dtype=bf)
    EREL[dcore, dsg, (pos & 127), eb[order] + (pos >> 7)] = \
        prel[order].astype(bf)
    ERAT = np.zeros((M_CORES, NSG, 128, NBLKmax), dtype=bf)
    ERAT[dcore, dsg, (pos & 127), eb[order] + (pos >> 7)] = \
        r32[order].astype(bf)
    eidx_in = _pack16(EIDX)                         # [M, NSG, 128, NBLKmax*8]

    # ---- phase 2 buckets: (core, sg, segment, tile-in-sg)
    v32 = np.asarray(v).astype(np.int32)
    j32 = np.asarray(j).astype(np.int32)
    NOUT = v32.shape[0]
    core2 = j32 // UC
    u2 = j32 - core2 * UC
    t2 = u2 >> 7
    jr = (u2 & 127).astype(np.float32)
    sg2 = t2 // SG
    tl2 = t2 % SG
    seg = v32 // SEGW
    vr = (v32 - seg * SEGW).astype(np.int16)
    bkt2 = ((core2 * NSG + sg2) * NSEG + seg) * SG + tl2
    NB2tot = M_CORES * NSG * NSEG * SG
    order2 = np.argsort(bkt2, kind="stable")
    b2s = bkt2[order2]
    cnts2 = np.bincount(bkt2, minlength=NB2tot)
    st2 = np.zeros(NB2tot + 1, np.int64)
    np.cumsum(cnts2, out=st2[1:])
    pos2 = np.arange(NOUT, dtype=np.int64) - st2[b2s]

    cnts2r = cnts2.reshape(M_CORES, NSG, NSEG, SG)
    cap2 = ((cnts2r.max(0) + 127) // 128) * 128     # [NSG, NSEG, SG]
    B2 = cap2 >> 7
    RS = cap2.sum(axis=2)
    RB = B2.sum(axis=2)
    tloff = np.zeros((NSG, NSEG, SG + 1), np.int64)
    np.cumsum(cap2, axis=2, out=tloff[:, :, 1:])
    regoff = np.zeros(NSG * NSEG + 1, np.int64)
    np.cumsum(RS.reshape(-1), out=regoff[1:])
    regblk = np.zeros(NSG * NSEG + 1, np.int64)
    np.cumsum(RB.reshape(-1), out=regblk[1:])
    TOT2S = int(regoff[-1])

    sg2s = sg2[order2]
    segs = seg[order2]
    tl2s = tl2[order2]
    c2s = core2[order2]
    base_s = regoff[sg2s * NSEG + segs] + tloff[sg2s, segs, tl2s]
    pslot = base_s + pos2
    PV = np.zeros((M_CORES, TOT2S), np.int16)
    PV[c2s, pslot] = vr[order2]
    PJR = np.full((M_CORES, 1, TOT2S), -1.0, dtype=bf)
    PJR[c2s, 0, pslot] = jr[order2].astype(bf)
    pv_in = _pack16(PV)                             # [M, 128, TOT2S//16]

    # res address per pair (sorted order -> scatter to original)
    rescol_s = regblk[sg2s * NSEG + segs] + \
        ((tloff[sg2s, segs, tl2s] + pos2) >> 7)
    respart_s = pos2 & 127
    rescol = np.empty(NOUT, np.int64)
    rescol[order2] = rescol_s
    respart = np.empty(NOUT, np.int64)
    respart[order2] = respart_s

    Vb = np.ascontiguousarray(np.asarray(V, dtype=np.float32)).astype(bf)
    Wf = np.asarray(W, dtype=np.float32)
    Wb = np.ascontiguousarray(0.5 * Wf).astype(bf)  # phase-2 table = W/2
    muf = np.ascontiguousarray(np.asarray(mu).reshape(1, D), dtype=np.float32)
    bvec = np.asarray(b, dtype=np.float32).reshape(-1)
    rw = 0.5 * Wf.sum(axis=1) + bvec                # host correction term

    in_maps = []
    for c in range(M_CORES):
        in_maps.append({
            "V": Vb, "W": Wb, "mu": muf,
            "eidx": eidx_in[c], "erel": np.asarray(EREL[c]),
            "erat": np.asarray(ERAT[c]),
            "pv": pv_in[c], "pjr": PJR[c],
        })
    shapes = dict(capr=tuple(capr.reshape(-1).tolist()),
                  cap2=tuple(cap2.reshape(-1).tolist()))
    meta = dict(NOUT=NOUT, core2=core2, rescol=rescol, respart=respart,
                rw_v=rw[v32])
    return in_maps, shapes, meta


def _unshard(results, meta):
    res = np.stack([results[c]["res"] for c in range(M_CORES)])  # [M,128,B]
    out = res[meta["core2"], meta["respart"], meta["rescol"]] + meta["rw_v"]
    return out.astype(np.float32)


def run(u, i, r, m, v, j, V, mu, W, b, trace=False, trace_kwargs=None):
    """Full pipeline; returns (out, BassKernelResults)."""
    from concourse import bass_utils

    in_maps, shapes, meta = _prep_inputs(u, i, r, m, v, j, V, mu, W, b)
    key = (shapes["capr"], shapes["cap2"])
    nc = _PROGRAM_CACHE.get(key)
    if nc is None:
        nc = _build_program(shapes)
        _PROGRAM_CACHE[key] = nc
    res = bass_utils.run_bass_kernel_spmd(
        nc, in_maps, list(range(M_CORES)), trace=trace, **(trace_kwargs or {}))
    return _unshard(res.results, meta), res


def kernel(u, i, r, m, v, j, V, mu, W, b):
    out, _ = run(u, i, r, m, v, j, V, mu, W, b, trace=False)
    return out
